# revision 50
# baseline (speedup 1.0000x reference)
"""Graphormer attention (N=2048, D=512, H=8 heads of 64) on 8 NeuronCores.

Strategy (tensor-parallel over heads, one head per core):
  - Host slices Q/K/V/O projection weights per head, transposes x once.
  - The z-bin bias is folded in multiplicatively: the per-head bias table is
    tiny (16 entries), so host precomputes W = exp(z_table[bin(z)]) transposed
    to the kernel's [key, query] layout, shipped as fp16.
  - On device (per core): fused Q^T/K^T projection (softmax scale folded into
    Wq on host), V projection, then a software-pipelined loop over key tiles:
    S^T = K^T-tiles x Q^T (fp32, PSUM), exp on ScalarE -> fp16,
    P = exp(S) * W on VectorE, O'^T = sum_k V'[k,65] x P (65th V column = 1
    => row 64 of O' is the softmax denominator Z), Y^T = Wo_h^T-tiles x O^T.
  - The loop cadence is bound by ScalarE exp ([128,1024] ~1.15us each); all
    matmuls, DMA and DVE multiplies hide underneath it.  Prologue evacuations
    (QK/V PSUM->SBUF) are spliced between the first few DVE multiplies, and
    Y^T for query chunk 0 is interleaved into chunk 1's loop.
  - Host divides each head's partial Y by its Z, sums heads, adds bias terms.
"""

import numpy as np
import ml_dtypes
from contextlib import ExitStack

import concourse.bass as bass
import concourse.tile as tile
from concourse import bacc, mybir
from concourse import bass_utils

N = 2048
D = 512
H = 8
HD = 64
NUM_Z_BINS = 16
MAX_Z = 5.0
SCALE = HD ** -0.5
NCORES = 8
QL = 1024          # query-chunk length (PSUM budget)
QC = N // QL       # 2 query chunks
KT = N // 128      # 16 key tiles

FP32 = mybir.dt.float32
FP16 = mybir.dt.float16
FP8 = mybir.dt.float8e4
FP8_NP = ml_dtypes.float8_e4m3fn
BF16 = mybir.dt.bfloat16
BF16_NP = ml_dtypes.bfloat16
FP16_NP = np.float16

AF = mybir.ActivationFunctionType
OP = mybir.AluOpType

_PROGRAM_CACHE = {}


def _build_program():
    if "nc" in _PROGRAM_CACHE:
        return _PROGRAM_CACHE["nc"]

    nc = bacc.Bacc(
        "TRN2",
        target_bir_lowering=False,
        debug=False,
        enable_asserts=False,
        num_devices=NCORES,
    )

    # host-packed layouts so every DMA has >=4KB contiguous lines:
    #   xT:  [128, (h=2, c=4, n=1024)] -- x^T chunked by query half/contraction
    #   wqv: [128, (c=4)*128 qk | (c=4)*64 v] -- per-head projection weights
    #   wt:  [(qc=2, tp=8)*128, 2048] -- bias tiles in key-tile pairs
    xT = nc.dram_tensor("xT", [128, 2 * D // 128 * QL], BF16,
                        kind="ExternalInput").ap()
    wqv = nc.dram_tensor("wqv", [128, D + D // 2 + D], BF16,
                         kind="ExternalInput").ap()
    wt = nc.dram_tensor("wt", [N // 2, 2 * N], FP16, kind="ExternalInput").ap()

    ypT = nc.dram_tensor("ypT", [D, N], FP16, kind="ExternalOutput").ap()
    zrow = nc.dram_tensor("zrow", [N], FP16, kind="ExternalOutput").ap()

    with tile.TileContext(nc) as tc:
        with ExitStack() as ctx:
            _emit(ctx, tc, xT, wqv, wt, ypT, zrow)
    nc.compile()
    _PROGRAM_CACHE["nc"] = nc
    return nc


def _emit(ctx, tc, xT, wqv, wt, ypT, zrow):
    nc = tc.nc
    CH = D // 128  # 4 contraction chunks of the model dim

    singles = ctx.enter_context(tc.tile_pool(name="singles", bufs=1))
    # PSUM budget is 16KB/partition (8 banks). ps_a slots are [128,1024]fp32
    # (2 banks x 3 slots = 6 banks) shared by the qk-proj/S/Y phases; ps_o
    # (2 banks) holds the V-projection scratch then the O' accumulator.
    ps_a = ctx.enter_context(tc.tile_pool(name="ps_a", bufs=3, space="PSUM"))
    ps_o = ctx.enter_context(tc.tile_pool(name="ps_o", bufs=1, space="PSUM"))
    wpool = ctx.enter_context(tc.tile_pool(name="wpool", bufs=6))
    epool = ctx.enter_context(tc.tile_pool(name="epool", bufs=7))
    ppool = ctx.enter_context(tc.tile_pool(name="ppool", bufs=7))
    ypool = ctx.enter_context(tc.tile_pool(name="ypool", bufs=4))

    # ---- constants + x^T load ------------------------------------------
    # wqv first (needed by the first QK matmul), then x^T in 256KB
    # per-chunk pieces so completion semaphores fire progressively, with
    # the first W pair spliced between the two x^T halves.
    # wqv carries wqk | wv | wo (wo bitcast into bf16 lanes, rows 0..63)
    wqv_sb = singles.tile([128, D + D // 2 + D], BF16)
    nc.sync.dma_start(out=wqv_sb, in_=wqv)
    wqk_sb = wqv_sb[:, 0:D]
    wv_sb = wqv_sb[:, D:D + D // 2]
    wo_sb = wqv_sb[0:HD, D + D // 2:].bitcast(FP16)

    xT_sb = singles.tile([128, 2 * CH * QL], BF16)

    def xs(g0, c, width):
        # x^T chunk c, global query/key columns [g0, g0+width) (one h-half)
        h, off = divmod(g0, QL)
        base = (h * CH + c) * QL + off
        return xT_sb[:, base:base + width]

    def load_x(h, cp):
        lo = (h * CH + 2 * cp) * QL
        nc.sync.dma_start(out=xT_sb[:, lo:lo + 2 * QL],
                          in_=xT[:, lo:lo + 2 * QL])

    load_x(0, 0)
    load_x(0, 1)

    w_quads = {}

    def issue_w(qc, tq):
        w_tile = wpool.tile([128, 4 * QL], FP16, tag="w")
        nc.sync.dma_start(
            out=w_tile,
            in_=wt[(qc * 4 + tq) * 128:(qc * 4 + tq + 1) * 128, :],
        )
        w_quads[(qc, tq)] = w_tile

    issue_w(0, 0)
    load_x(1, 0)
    load_x(1, 1)
    issue_w(0, 1)
    issue_w(0, 2)
    issue_w(0, 3)
    issue_w(1, 0)

    # ---- PE warm-up: dummy matmuls ramp the p-state while DMA streams --
    scratch = singles.tile([128, 512], BF16)
    nc.vector.memset(scratch, 0.0)
    wu = ps_a.tile([128, QL], FP32, tag="big")
    for _ in range(13):
        nc.tensor.matmul(wu[:, 0:512], lhsT=scratch[:, 0:128], rhs=scratch,
                         start=True, stop=True)

    # ---- fused Q^T/K^T projection: one [128,128] weight block computes
    # (scaled) Q^T into PSUM rows 0-63 and K^T into rows 64-127.  The
    # matmul requires both operands at the same base partition, so the
    # evacuation splits into separate q/k tiles (Q on ACT while it's idle
    # pre-exp, the rest on DVE).
    qT_sb = singles.tile([HD, N], BF16)
    kT_sb = singles.tile([HD, N], BF16)
    qk_pending = {}

    def emit_qk_mm(jp, cs=range(CH)):
        # c-major so each matmul is gated only by its own x chunk's DMA;
        # jp=1 is emitted in two chunks spliced into the loop.
        if 0 in cs:
            pool = ps_a if jp == 0 else ps_o
            tag = "big" if jp == 0 else "ot"
            pt = pool.tile([128, QL], FP32, tag=tag)
            qk_pending[jp] = pt
        pt = qk_pending[jp]
        for c in cs:
            for jj in range(2):
                nc.tensor.matmul(
                    pt[:, jj * 512:(jj + 1) * 512],
                    lhsT=wqk_sb[:, c * 128:(c + 1) * 128],
                    rhs=xs(jp * QL + jj * 512, c, 512),
                    start=(c == 0),
                    stop=(c == CH - 1),
                )

    def evac_qk(jp, q_on_act, k_only=None):
        # q-half on ACT while it's idle pre-exp; k-half on DVE (it gates
        # the S pipeline).  k_only True/False lets the two halves be
        # spliced into different loop iterations.
        pt = qk_pending[jp]
        dst = slice(jp * QL, (jp + 1) * QL)
        if k_only is not True:
            qk_pending.pop(jp)
            if q_on_act:
                nc.scalar.copy(qT_sb[:, dst], pt[0:HD, :])
            else:
                nc.vector.tensor_copy(qT_sb[:, dst], pt[0:HD, :])
        if k_only is not False:
            nc.vector.tensor_copy(kT_sb[:, dst], pt[HD:128, :])

    # ---- S tile emission ------------------------------------------------
    pending = {}

    def emit_s(qc, t):
        st = ps_a.tile([128, QL], FP32, tag="big")
        for n in range(QL // 512):
            nc.tensor.matmul(
                st[:, n * 512:(n + 1) * 512],
                lhsT=kT_sb[:, t * 128:(t + 1) * 128],
                rhs=qT_sb[:, qc * QL + n * 512: qc * QL + (n + 1) * 512],
                start=True,
                stop=True,
            )
        pending[(qc, t)] = st

    # ---- V projection: V' = [k-tile 128, 65] per tile, col 64 = 1.0 ----
    v_sb = singles.tile([128, KT * (HD + 1)], FP16)
    nc.vector.memset(v_sb, 1.0)
    v_pending = {}

    def emit_v_mm(half, quarter=None):
        # half 0 uses the ps_o banks (freed before the O' accumulator is
        # first written); half 1 rides the ps_a rotation, emitted in two
        # quarters spliced into the loop.
        if half == 0 or quarter == 0:
            vp = ps_o.tile([128, QL], FP32, tag="ot")
            v_pending[half] = vp
        else:
            vp = v_pending[half]
        mms = range(KT // 2) if quarter is None else             range(quarter * (KT // 4), (quarter + 1) * (KT // 4))
        for mm in mms:
            m = half * (KT // 2) + mm
            for c in range(CH):
                nc.tensor.matmul(
                    vp[:, mm * HD:(mm + 1) * HD],
                    lhsT=xs(m * 128, c, 128),
                    rhs=wv_sb[:, c * HD:(c + 1) * HD],
                    start=(c == 0),
                    stop=(c == CH - 1),
                )

    def evac_v(half, quarter=None):
        vp = v_pending[half]
        if quarter is None:
            lo, n = 0, KT // 2
        else:
            lo, n = quarter * (KT // 4), KT // 4
        mlo = half * (KT // 2) + lo
        nc.vector.tensor_copy(
            v_sb.rearrange("p (t c) -> p t c", c=HD + 1)
                [:, mlo:mlo + n, 0:HD],
            vp[:, lo * HD:(lo + n) * HD].rearrange("p (t c) -> p t c", c=HD),
        )

    # The whole projection prologue runs before the loop: QK(0) (ps_a),
    # QK(1) (ps_o, free until the O' accumulator), V halves, and all
    # evacuations.  The loop then has a clean steady state.
    emit_qk_mm(0)
    evac_qk(0, q_on_act=True)
    emit_s(0, 0)
    emit_s(0, 1)
    emit_qk_mm(1)
    emit_v_mm(0)
    evac_qk(1, q_on_act=False, k_only=True)
    evac_qk(1, q_on_act=True, k_only=False)
    emit_s(0, 2)
    emit_s(0, 3)
    evac_v(0)
    emit_v_mm(1, quarter=0)
    emit_v_mm(1, quarter=1)
    evac_v(1, 0)
    evac_v(1, 1)

    oT_sb = singles.tile([HD + 1, N], FP16)

    def emit_y(n2, m, tail):
        # Y^T block for query columns [n2*1024, (n2+1)*1024), model rows
        # [m*128, (m+1)*128).  Evacuations go to DVE mid-loop (ACT is the
        # cadence-critical engine); tail blocks alternate ACT/DVE and are
        # evacuated/DMA'd per 512-column half to shorten the drain.
        yt = ps_a.tile([128, QL], FP32, tag="big")
        for nl in range(2):
            n = n2 * 2 + nl
            nc.tensor.matmul(
                yt[:, nl * 512:(nl + 1) * 512],
                lhsT=wo_sb[:, m * 128:(m + 1) * 128],
                rhs=oT_sb[0:HD, n * 512:(n + 1) * 512],
                start=True,
                stop=True,
            )
        y_sb = ypool.tile([128, QL], FP16, tag="ysb")
        if tail:
            # split evacuation across ACT/DVE halves, but one full-block
            # DMA (2KB lines; finer splits degrade to 1KB descriptors),
            # drained over both HWDGE rings in parallel
            for nl in range(2):
                half = slice(nl * 512, (nl + 1) * 512)
                if (m + nl) % 2 == 0:
                    nc.scalar.copy(y_sb[:, half], yt[:, half])
                else:
                    nc.vector.tensor_copy(y_sb[:, half], yt[:, half])
        else:
            nc.vector.tensor_copy(y_sb, yt)
        nc.sync.dma_start(
            out=ypT[m * 128:(m + 1) * 128, n2 * QL:(n2 + 1) * QL],
            in_=y_sb,
        )

    # ---- main loop: exp -> *W -> PV with S(t+4)/W-quad prefetch ---------
    pairs = {}
    for qc in range(QC):
        ot = ps_o.tile([HD + 1, QL], FP32, tag="ot")
        for t in range(KT):
            gt = qc * KT + t          # global tile index 0..31
            # Y blocks for query chunk 0 ride inside chunk 1's loop; the
            # yt allocation is emitted before emit_s so its PSUM-rotation
            # stall does not delay the S pipeline.
            ta = gt + 4
            if ta < QC * KT:
                emit_s(ta // KT, ta % KT)
            if qc == 1 and t in (1, 4, 7, 10):
                emit_y(0, (t - 1) // 3, tail=False)
            if gt % 4 == 0:
                pq = gt // 4 + 5
                if pq < QC * KT // 4:
                    issue_w(pq // 4, pq % 4)
            st = pending.pop((qc, t))
            wp = w_quads[(qc, t // 4)]
            if t % 2 == 0:
                e_pair = epool.tile([128, 2 * QL], FP16, tag="e")
                p_pair = ppool.tile([128, 2 * QL], FP16, tag="p")
                pairs[(qc, t)] = (e_pair, p_pair)
            else:
                e_pair, p_pair = pairs.pop((qc, t - 1))
            eh = slice((t % 2) * QL, (t % 2 + 1) * QL)
            nc.scalar.activation(e_pair[:, eh], st, AF.Exp)
            if t % 2 == 1:
                # one DVE multiply covers the whole pair (W quad layout
                # keeps the two tiles' bias columns adjacent)
                h2 = slice((t // 2 % 2) * 2 * QL, (t // 2 % 2 + 1) * 2 * QL)
                nc.vector.tensor_mul(p_pair, e_pair, wp[:, h2])
            if t % 4 == 3:
                del w_quads[(qc, t // 4)]
            if t % 2 == 1:
                for tt in (t - 1, t):
                    for n in range(QL // 512):
                        lo = (tt % 2) * QL + n * 512
                        nc.tensor.matmul(
                            ot[:, n * 512:(n + 1) * 512],
                            lhsT=v_sb[:, tt * (HD + 1):(tt + 1) * (HD + 1)],
                            rhs=p_pair[:, lo:lo + 512],
                            start=(tt == 0),
                            stop=(tt == KT - 1),
                        )
        if qc == 0:
            nc.vector.tensor_copy(oT_sb[:, 0:QL], ot)
        else:
            # halves on ACT (idle after the last exp): the first half
            # unblocks the Y matmuls while the second is still copying
            nc.scalar.copy(oT_sb[:, QL:QL + 512], ot[:, 0:512])
            nc.scalar.copy(oT_sb[:, QL + 512:N], ot[:, 512:QL])
        nc.sync.dma_start(
            out=zrow.rearrange("(a n) -> a n", a=1)[:, qc * QL:(qc + 1) * QL],
            in_=oT_sb[HD:HD + 1, qc * QL:(qc + 1) * QL])

    wu2 = ps_a.tile([128, QL], FP32, tag="big")
    for _ in range(6):
        nc.tensor.matmul(wu2[:, 0:512], lhsT=scratch[:, 0:128], rhs=scratch,
                         start=True, stop=True)
    for m in range(D // 128):
        emit_y(1, m, tail=True)


def _install_ntff_hook():
    """Recreate the missing ``antenv.axon_hooks`` module so that
    run_bass_kernel_spmd(trace=True) can capture NTFF profiles via the
    libaxon_pjrt.so ctypes hook (see trn_agent_boot.trn_boot)."""
    import sys
    import types

    try:
        import antenv.axon_hooks  # noqa: F401
        return
    except ImportError:
        pass
    import antenv
    from trn_agent_boot.trn_boot import _ntff_profile_via_ctypes

    mod = types.ModuleType("antenv.axon_hooks")
    mod._hook = _ntff_profile_via_ctypes("/opt/axon/libaxon_pjrt.so")
    mod.set_axon_ntff_profile_hook = lambda h: setattr(mod, "_hook", h)
    mod.get_axon_ntff_profile_hook = lambda: mod._hook
    sys.modules["antenv.axon_hooks"] = mod
    antenv.axon_hooks = mod
    # keep profile artifacts local; the sandbox has no bucket access
    bass_utils.upload_artifacts = lambda tmpdir: tmpdir


def kernel(x, z_matrix, Wq, bq, Wk, bk, Wv, bv, Wo, bo, z_table, _trace=False):
    if _trace:
        _install_ntff_hook()
    x = np.ascontiguousarray(np.asarray(x, dtype=np.float32))
    z_matrix = np.asarray(z_matrix, dtype=np.float32)
    Wq = np.asarray(Wq, dtype=np.float32)
    Wk = np.asarray(Wk, dtype=np.float32)
    Wv = np.asarray(Wv, dtype=np.float32)
    Wo = np.asarray(Wo, dtype=np.float32)
    bq = np.asarray(bq, dtype=np.float32)
    bk = np.asarray(bk, dtype=np.float32)
    bv = np.asarray(bv, dtype=np.float32)
    bo = np.asarray(bo, dtype=np.float32)
    z_table = np.asarray(z_table, dtype=np.float32)

    nc = _build_program()

    # pack x^T as [128, (h, c, n)] so each DMA has 8KB contiguous lines
    xTp = np.ascontiguousarray(
        x.T.reshape(4, 128, 2, 1024).transpose(1, 2, 0, 3).reshape(128, 8192)
    ).astype(BF16_NP)
    binsT = np.clip(
        np.floor(z_matrix.T / MAX_Z * NUM_Z_BINS).astype(np.int32), 0, NUM_Z_BINS - 1
    )
    exp_tab = np.exp(z_table)  # [16, H] fp32

    in_maps = []
    for h in range(NCORES):
        sl = slice(h * HD, (h + 1) * HD)
        wt_h = exp_tab[:, h][binsT]  # [key, query] layout
        if bq[sl].any() or bk[sl].any():
            # logits = scale*(q+bq).(k+bk); per-query terms cancel in
            # softmax, leaving a per-key multiplicative factor.
            key_term = SCALE * ((x @ Wk[:, sl] + bk[sl]) @ bq[sl])  # [N]
            wt_h = wt_h * np.exp(key_term)[:, None]
        # key-tile quads: [(qc, tq)*128, (tl, n)] with 8KB lines
        wt_h = np.ascontiguousarray(
            wt_h.reshape(4, 4, 128, 2, 1024).transpose(3, 0, 2, 1, 4)
            .reshape(1024, 4096)
        ).astype(FP16_NP)
        wqk_h = np.concatenate([Wq[:, sl] * SCALE, Wk[:, sl]], axis=1)
        wo_fp16 = np.ascontiguousarray(Wo[sl, :]).astype(FP16_NP)  # [64, 512]
        wo_bits = np.zeros((128, 512), dtype=BF16_NP)
        wo_bits[0:HD, :] = wo_fp16.view(BF16_NP)
        wqv_h = np.concatenate([
            wqk_h.reshape(4, 128, 128).transpose(1, 0, 2).reshape(128, 512),
            Wv[:, sl].reshape(4, 128, 64).transpose(1, 0, 2).reshape(128, 256),
            wo_bits,
        ], axis=1)
        in_maps.append({
            "xT": xTp,
            "wqv": np.ascontiguousarray(wqv_h).astype(BF16_NP),
            "wt": wt_h,
        })

    res = bass_utils.run_bass_kernel_spmd(
        nc, in_maps, core_ids=list(range(NCORES)), trace=_trace,
    )

    acc = np.zeros((D, N), dtype=np.float64)
    for h in range(NCORES):
        ypT_h = res.results[h]["ypT"].astype(np.float64)
        z_h = res.results[h]["zrow"].astype(np.float64)
        acc += ypT_h / z_h[None, :]
    out = acc.T + (bv @ Wo)[None, :] + bo[None, :]
    out_f32 = out.astype(np.float32)
    if _trace:
        return out_f32, res
    return out_f32
